# revision 13
# baseline (speedup 1.0000x reference)
"""Trainium2 Bass kernel for Conv2D_DT (distance-transform conv).

d(n,o,h,w) = || patch(n,:,h,w) - W[o,:] ||_2  with 3x3/pad1 im2col patches.

Strategy (8 NeuronCores, data-parallel over batch):
  - 4 images per core, processed as 2 pairs: image A on SBUF partitions
    0-63, image B on partitions 64-127 (channels = partition dim).
  - d2 = ||p||^2 + ||w||^2 - 2 p.w  accumulated fully in PSUM:
      * 9 shifted matmuls (taps) with lhsT = -2*W_tap  [K=64 per image]
      * 3 matmuls with lhsT = ones over t = 3-column box sum of x^2
        (so the 3 row-taps of t complete the 3x3 box sum of x^2 and the
        ones-contraction sums it over channels)
  - The two images' K=64 matmuls land on PE row-groups (0,0)/(64,0) and
    run concurrently -> full 128-row array utilization.
  - rhs streams as float32r (full PE rate at N>=256); lhsT is bf16
    (enables fast weight load); PSUM accumulates fp32.
  - epilogue: one ScalarE op  out = Sqrt(psum + w2[o])  then DMA out.
    (d2 >= ~200 for this data distribution, so Sqrt never sees <0.)
"""

import sys

_REPO = "/opt/trn_rl_repo"
if _REPO not in sys.path:
    sys.path.insert(0, _REPO)

import ml_dtypes
import numpy as np

import concourse.bass as bass  # noqa: F401  (AP types used via handles)
import concourse.mybir as mybir
import concourse.tile as tile
from concourse import bacc
from concourse.bass_utils import run_bass_kernel_spmd

# Problem geometry (hardcoded per harness contract).
N, C, H, W_DIM, O = 32, 64, 56, 56, 128
NCORES = 8
NL = N // NCORES  # images per core
NPAIR = NL // 2  # image pairs per core
HP = WP = 58  # zero-padded spatial dims
RCH = 8  # output rows per PSUM chunk
NCH = H // RCH  # 7 chunks per image
NSLOT = 12  # 9 x-taps + 3 t-taps

F32 = mybir.dt.float32
F32R = mybir.dt.float32r
BF16 = mybir.dt.bfloat16

_PROGRAM = None


def _build_program():
    nc = bacc.Bacc(
        "TRN2",
        target_bir_lowering=False,
        debug=False,
        enable_asserts=False,
        num_devices=NCORES,
    )
    xs = nc.dram_tensor("xs", [NL, C, HP, WP], F32R, kind="ExternalInput")
    lw = nc.dram_tensor("lw", [128, NSLOT, 128], F32R, kind="ExternalInput")
    w2 = nc.dram_tensor("w2", [128, 1], F32, kind="ExternalInput")
    out = nc.dram_tensor("out", [NL, O, H, W_DIM], F32, kind="ExternalOutput")

    with tile.TileContext(nc) as tc:
        with (
            tc.tile_pool(name="const", bufs=1) as cpool,
            tc.tile_pool(name="imgs", bufs=2) as ipool,
            tc.tile_pool(name="outs", bufs=4) as opool,
            tc.tile_pool(name="psum", bufs=6, space="PSUM") as ppool,
        ):
            lwt = cpool.tile([128, NSLOT, 128], F32R)
            nc.sync.dma_start(out=lwt[:], in_=lw[:, :, :])
            w2t = cpool.tile([128, 1], F32)
            nc.sync.dma_start(out=w2t[:], in_=w2[:, :])

            for p in range(NPAIR):
                na, nb = 2 * p, 2 * p + 1
                xp = ipool.tile([128, HP, WP], F32R, tag="xp")
                # input arrives host-side zero-padded to [C, 58, 58]
                nc.sync.dma_start(out=xp[0:64, :, :], in_=xs[na])
                nc.sync.dma_start(out=xp[64:128, :, :], in_=xs[nb])

                # t = 3-col box sum of x^2 (per channel), [128, 58, 56]
                sq = ipool.tile([128, HP, WP], F32, tag="sq")
                nc.scalar.activation(
                    out=sq[:], in_=xp[:], func=mybir.ActivationFunctionType.Square
                )
                u = ipool.tile([128, HP, W_DIM], F32, tag="u")
                nc.vector.tensor_add(u[:], sq[:, :, 0:56], sq[:, :, 1:57])
                tt = ipool.tile([128, HP, W_DIM], F32R, tag="tt")
                nc.vector.tensor_add(tt[:], u[:], sq[:, :, 2:58])

                for ch in range(NCH):
                    h0 = ch * RCH
                    psa = ppool.tile([128, RCH, W_DIM], F32, tag="ps")
                    psb = ppool.tile([128, RCH, W_DIM], F32, tag="ps")
                    for slot in range(NSLOT):
                        if slot < 9:
                            kh, kw = divmod(slot, 3)
                            rhs = xp[:, h0 + kh : h0 + kh + RCH, kw : kw + 56]
                        else:
                            kh = slot - 9
                            rhs = tt[:, h0 + kh : h0 + kh + RCH, 0:56]
                        st, sp = slot == 0, slot == NSLOT - 1
                        nc.tensor.matmul(
                            psa[:],
                            lwt[0:64, slot, :],
                            rhs[0:64],
                            start=st,
                            stop=sp,
                        )
                        nc.tensor.matmul(
                            psb[:],
                            lwt[64:128, slot, :],
                            rhs[64:128],
                            start=st,
                            stop=sp,
                        )
                    for ps, n_img in ((psa, na), (psb, nb)):
                        ot = opool.tile([128, RCH, W_DIM], F32, tag="ot")
                        nc.scalar.activation(
                            out=ot[:],
                            in_=ps[:],
                            func=mybir.ActivationFunctionType.Sqrt,
                            bias=w2t[:],
                            scale=1.0,
                        )
                        nc.sync.dma_start(
                            out=out[n_img, :, h0 : h0 + RCH, :], in_=ot[:]
                        )
    nc.compile()
    return nc


def _host_weights(W):
    """lhsT tiles [128, 12, 128] bf16 (dup on both partition halves) + w2."""
    W = np.asarray(W, np.float32)
    lhs = np.zeros((128, NSLOT, 128), np.float32)
    cidx = np.arange(C)
    for kh in range(3):
        for kw in range(3):
            slot = kh * 3 + kw
            blk = (-2.0 * W[:, cidx * 9 + kh * 3 + kw]).T  # [C, O]
            lhs[0:64, slot, :] = blk
            lhs[64:128, slot, :] = blk
    lhs[:, 9:12, :] = 1.0  # ones: channel-sum of t (the ||p||^2 term)
    w2 = (W * W).sum(axis=1).astype(np.float32).reshape(128, 1)
    return lhs, w2


def get_program():
    global _PROGRAM
    if _PROGRAM is None:
        _PROGRAM = _build_program()
    return _PROGRAM


def make_in_maps(x, W):
    x = np.asarray(x, np.float32)
    xpad = np.zeros((N, C, HP, WP), np.float32)
    xpad[:, :, 1 : H + 1, 1 : W_DIM + 1] = x
    lhs, w2 = _host_weights(W)
    return [
        {"xs": xpad[i * NL : (i + 1) * NL], "lw": lhs, "w2": w2}
        for i in range(NCORES)
    ]


def kernel(x, W):
    nc = get_program()
    in_maps = make_in_maps(x, W)
    res = run_bass_kernel_spmd(nc, in_maps, list(range(NCORES)))
    outs = [res.results[i]["out"] for i in range(NCORES)]
    return np.concatenate(outs, axis=0)
